# revision 10
# baseline (speedup 1.0000x reference)
"""Trainium2 Bass kernel for 3-hop GCN (nn_GNN_70978629534135).

Strategy (8 NeuronCores, SPMD):
  - Nodes are permuted by in-degree (descending) and snake-dealt across the 8
    cores so every core owns NL=12544 slots (12500 real + 44 zero "fake" pads)
    with a near-identical degree profile.
  - All edge indices are remapped into the permuted id space on the host, and
    partitioned by destination core.  For each 128-node tile the in-edges are
    padded to the tile-max degree K_t and stored as a [128, K_t] int32 gather
    table (pad slots point at a row that is always zero).
  - Since matmul distributes over the neighbor sum, each hop gathers raw h
    (not h@W): per-k indirect DMAs (128 rows each, the HW limit for dynamic
    offsets), one wide strided DVE op per tree level for the neighbor sum,
    PE transpose, matmul, ReLU.
  - Encoder/decoder matmul chains are arranged transposed (lhsT=weight) so no
    PE transposes are needed there; biases that are all-zero on the host are
    elided entirely (this model's biases are structurally zero).
  - log_softmax: per-tile max/exp (Exp table stays loaded), one batched Ln at
    the end, then a broadcast-add pass writes the output.
  - AllGathers are split in half so the first half overlaps the tail of each
    hop's compute.

kernel(**inputs) takes the FULL unsharded inputs and returns the FULL output.
"""

import os
import numpy as np

N, E, F, H, D, R, C, W = 100_000, 600_000, 128, 256, 128, 3, 40, 8
P = 128


# ---------------------------------------------------------------- host prep
def _build_plan(edge_index, n, w, nl):
    tiles = nl // P
    half = (tiles // 2) * P  # AG split point (slots per core)
    src = edge_index[0].astype(np.int64)
    dst = edge_index[1].astype(np.int64)
    deg = np.bincount(dst, minlength=n)
    order = np.argsort(-deg, kind="stable")
    new_of_old = np.empty(n, dtype=np.int64)
    pos = np.arange(n)
    new_of_old[order] = (pos % w) * nl + (pos // w)
    src_n = new_of_old[src]
    dst_n = new_of_old[dst]

    # hfull row layout is split for the two-phase AllGather:
    #   rows [0, w*half)        = slot s < half of core c at row c*half + s
    #   rows [w*half, w*nl)     = slot s >= half at w*half + c*(nl-half) + (s-half)
    def hrow(gid):
        c, s = gid // nl, gid % nl
        return np.where(s < half, c * half + s,
                        w * half + c * (nl - half) + (s - half))
    src_h = hrow(src_n)
    zrow = int(hrow(np.array([nl - 1]))[0])  # core 0's last slot: fake (zero)

    per_core = []
    Ks = np.zeros(tiles, dtype=np.int64)
    csr = []
    for c in range(w):
        m = (dst_n >= c * nl) & (dst_n < (c + 1) * nl)
        d_loc = dst_n[m] - c * nl
        s_glob = src_h[m]
        o = np.argsort(d_loc, kind="stable")
        d_loc, s_glob = d_loc[o], s_glob[o]
        counts = np.bincount(d_loc, minlength=nl)
        rowptr = np.concatenate([[0], np.cumsum(counts)])
        csr.append((d_loc, s_glob, rowptr))
        np.maximum(Ks, counts.reshape(tiles, P).max(axis=1), out=Ks)

    offs = np.concatenate([[0], np.cumsum(Ks)]).astype(np.int64)
    sumk = int(Ks.sum())
    for c in range(w):
        d_loc, s_glob, rowptr = csr[c]
        rank = np.arange(len(d_loc)) - rowptr[d_loc]
        col = offs[d_loc // P] + rank
        idx = np.full((P, sumk), zrow, dtype=np.int32)
        idx[d_loc % P, col] = s_glob
        per_core.append(idx)

    old_of_new = np.full(w * nl, -1, dtype=np.int64)
    old_of_new[new_of_old] = np.arange(n)
    return new_of_old, old_of_new, per_core, Ks.astype(int), offs, sumk


# ------------------------------------------------------------- device program
def _emit(tc, io, cfg):
    import concourse.bass as bass
    from concourse import mybir
    from concourse.masks import make_identity

    nc = tc.nc
    f32 = mybir.dt.float32
    bf16 = mybir.dt.bfloat16
    nl, tiles, sumk = cfg["NL"], cfg["TILES"], cfg["SUMK"]
    Ks, offs = cfg["Ks"], cfg["offs"]
    h2, r_hops = cfg["H"], cfg["R"]
    ncls = cfg["C"]
    nz = cfg["NONZERO_BIAS"]  # dict name -> bool
    AG_GROUPS = [list(range(cfg["W"]))]
    ADD = mybir.AluOpType.add
    Relu = mybir.ActivationFunctionType.Relu
    HALF = (tiles // 2) * P  # AG split point (rows)

    def rank1(psum_ap, ones, bias_ap, stop=True):
        nc.tensor.matmul(psum_ap, lhsT=ones, rhs=bias_ap, start=False, stop=stop)

    with tc.tile_pool(name="const", bufs=1) as cp, \
         tc.tile_pool(name="sb", bufs=4) as sb, \
         tc.tile_pool(name="sb_g", bufs=8) as sbg, \
         tc.tile_pool(name="sb_h", bufs=4) as sbh, \
         tc.tile_pool(name="ps", bufs=2, space="PSUM") as ps:

        def ptile(tag, width):
            return ps.tile([P, width], f32, space="PSUM", tag=tag, name=tag)

        ident = cp.tile([P, P], f32)
        make_identity(nc, ident[:])
        zeros128 = cp.tile([P, P], bf16)
        nc.gpsimd.memset(zeros128[:], 0.0)
        real_end = cfg["REAL_PER_CORE"]

        def store_h(dst_dram, t, h_tile):
            lo, hi = t * P, (t + 1) * P
            real_hi = min(hi, real_end)
            if real_hi > lo:
                nc.sync.dma_start(dst_dram[lo:real_hi, :], h_tile[0:real_hi - lo, :])
            if hi > real_hi:
                nc.sync.dma_start(dst_dram[real_hi:hi, :],
                                  zeros128[0:hi - real_hi, :])
        ones = cp.tile([1, P], bf16)
        nc.gpsimd.memset(ones[:], 1.0)

        # resident weights
        w1 = cp.tile([P, h2], bf16)          # enc_w1 [F, H]
        nc.sync.dma_start(w1[:], io["enc_w1"][:])
        w2 = cp.tile([P, h2], bf16)          # enc_w2 [H, D] -> [:, j*128:] = rows j
        for j in range(h2 // P):
            nc.sync.dma_start(w2[:, j * P:(j + 1) * P], io["enc_w2"][j * P:(j + 1) * P, :])
        gw = cp.tile([P, r_hops * P], bf16)  # gcn_w stacked [R*D, D]
        for r in range(r_hops):
            nc.sync.dma_start(gw[:, r * P:(r + 1) * P], io["gcn_w"][r * P:(r + 1) * P, :])
        dw1 = cp.tile([P, P], bf16)
        nc.sync.dma_start(dw1[:], io["dec_w1"][:])
        dw2 = cp.tile([P, ncls], bf16)
        nc.sync.dma_start(dw2[:], io["dec_w2"][:])
        if nz["enc_b1"]:
            b1T = cp.tile([P, h2 // P], bf16)
            nc.sync.dma_start(b1T[:], io["enc_b1T"][:])
        if nz["enc_b2"]:
            b2 = cp.tile([1, P], bf16)
            nc.sync.dma_start(b2[:], io["enc_b2"][:])
        if nz["gcn_b"]:
            gb = cp.tile([1, r_hops * P], bf16)
            nc.sync.dma_start(gb[:], io["gcn_b"][:])
            gb2T = cp.tile([P, 1], bf16)
            nc.sync.dma_start(gb2T[:], io["gcn_b2T"][:])
        if nz["dec_b1"]:
            db1T = cp.tile([P, 1], bf16)
            nc.sync.dma_start(db1T[:], io["dec_b1T"][:])
        if nz["dec_b2"]:
            db2 = cp.tile([1, ncls], bf16)
            nc.sync.dma_start(db2[:], io["dec_b2"][:])

        idx_sb = cp.tile([P, sumk], mybir.dt.int32)
        nc.sync.dma_start(idx_sb[:], io["idx"][:])

        # decoder softmax state (persists across tiles)
        logits_all = cp.tile([P, tiles * ncls], f32)
        nmx_all = cp.tile([P, tiles], f32)
        esum_all = cp.tile([P, tiles], f32)
        ln_all = cp.tile([P, tiles], f32)
        shift_all = cp.tile([P, tiles], f32)
        ex_scratch = cp.tile([P, ncls], f32)

        # internal DRAM: allgather bounce + full tables
        bounce = [nc.dram_tensor(f"bounce{r}", [nl, P], bf16, kind="Internal")
                  for r in range(r_hops)]
        hfull = [nc.dram_tensor(f"hfull{r}", [cfg["W"] * nl, P], bf16,
                                kind="Internal", addr_space="Shared")
                 for r in range(r_hops)]

        def allgather_split(r):
            # hfull row layout: [w*HALF first-half rows | w*(nl-HALF) rest]
            # so each half-AllGather writes one contiguous block; the first
            # half is issued as soon as rows [0, HALF) of bounce are stored.
            nc.gpsimd.collective_compute(
                "AllGather", mybir.AluOpType.bypass,
                ins=[bounce[r][0:HALF, :]],
                outs=[hfull[r][0:cfg["W"] * HALF, :]],
                replica_groups=AG_GROUPS)
            nc.gpsimd.collective_compute(
                "AllGather", mybir.AluOpType.bypass,
                ins=[bounce[r][HALF:nl, :]],
                outs=[hfull[r][cfg["W"] * HALF:cfg["W"] * nl, :]],
                replica_groups=AG_GROUPS)

        # ---------------- encoder ----------------
        # z1T = relu(w1.T @ xT [+ b1T]) computed directly in transposed layout;
        # h0 = z1 @ w2 [+ b2] via lhsT=z1T halves.  Two dst tiles are batched
        # per pass (256-wide rhs) to halve op counts.
        TB = 2
        for t0 in range(0, tiles, TB):
            tb = min(TB, tiles - t0)
            wb = tb * P
            xT_t = sb.tile([P, TB * P], bf16, name="xT_t")
            nc.sync.dma_start(xT_t[:, 0:wb], io["xT"][:, t0 * P:t0 * P + wb])
            p1a = ptile("psA", TB * P)
            p1b = ptile("psB", TB * P)
            nc.tensor.matmul(p1a[:, 0:wb], lhsT=w1[:, 0:P], rhs=xT_t[:, 0:wb],
                             start=True, stop=True)
            nc.tensor.matmul(p1b[:, 0:wb], lhsT=w1[:, P:h2], rhs=xT_t[:, 0:wb],
                             start=True, stop=True)
            z1a = sbh.tile([P, TB * P], bf16, name="z1a")
            z1b = sbh.tile([P, TB * P], bf16, name="z1b")
            if nz["enc_b1"]:
                nc.scalar.activation(z1a[:, 0:wb], p1a[:, 0:wb], Relu,
                                     bias=b1T[:, 0:1])
                nc.scalar.activation(z1b[:, 0:wb], p1b[:, 0:wb], Relu,
                                     bias=b1T[:, 1:2])
            else:
                nc.scalar.activation(z1a[:, 0:wb], p1a[:, 0:wb], Relu)
                nc.scalar.activation(z1b[:, 0:wb], p1b[:, 0:wb], Relu)
            for i in range(tb):
                t = t0 + i
                p2 = ptile("psC", P)
                nc.tensor.matmul(p2[:], lhsT=z1a[:, i * P:(i + 1) * P],
                                 rhs=w2[:, 0:P], start=True, stop=False)
                nc.tensor.matmul(p2[:], lhsT=z1b[:, i * P:(i + 1) * P],
                                 rhs=w2[:, P:h2],
                                 start=False, stop=not nz["enc_b2"])
                if nz["enc_b2"]:
                    rank1(p2[:], ones[:], b2[:])
                h0 = sbh.tile([P, P], bf16, name="h0")
                nc.vector.tensor_copy(h0[:], p2[:])
                store_h(bounce[0], t, h0)
        allgather_split(0)

        # ------------- gather + wide tree-sum ---------------------
        def gather_tile(src_full, t):
            K = int(Ks[t])
            off = int(offs[t])
            g = sbg.tile([P, max(K, 1) * P], bf16, name="g")
            for k in range(K):
                nc.gpsimd.indirect_dma_start(
                    out=g[:, k * P:(k + 1) * P], out_offset=None, in_=src_full[:],
                    in_offset=bass.IndirectOffsetOnAxis(
                        ap=idx_sb[:, off + k:off + k + 1], axis=0))
            return g

        def tree_sum(g, K):
            """Sum K gathered [128,128] blocks of g -> f32 tile [128, 128]."""
            sf = sbh.tile([P, max((K + 1) // 2, 1) * P], f32, name="sf")
            if K == 0:
                nc.vector.memset(sf[:, :P], 0.0)
                return sf
            if K == 1:
                nc.vector.tensor_copy(sf[:, :P], g[:, 0:P])
                return sf
            half = K // 2
            # level 1: one strided op: pairs (2j, 2j+1) -> sf (bf16+bf16->f32)
            gpair = g[:, 0:2 * half * P].rearrange(
                "p (k two j) -> p two k j", two=2, j=P)
            sf3 = sf[:, 0:half * P].rearrange("p (k j) -> p k j", j=P)
            nc.vector.tensor_tensor(sf3, gpair[:, 0], gpair[:, 1], op=ADD)
            if K % 2:
                nc.vector.tensor_copy(sf[:, half * P:(half + 1) * P],
                                      g[:, (K - 1) * P:K * P])
            kk = (K + 1) // 2
            while kk > 1:
                if kk % 2:
                    nc.vector.tensor_tensor(
                        sf[:, :P], sf[:, :P], sf[:, (kk - 1) * P:kk * P], op=ADD)
                    kk -= 1
                half2 = kk // 2
                nc.vector.tensor_tensor(
                    sf[:, :half2 * P], sf[:, :half2 * P],
                    sf[:, half2 * P:kk * P], op=ADD)
                kk = half2
            return sf

        def transpose_sb(src_ap, name):
            pt = ptile("psA", P)
            nc.tensor.transpose(pt[:], src_ap, ident[:])
            out = sbh.tile([P, P], bf16, name=f"sb_{name}")
            nc.vector.tensor_copy(out[:], pt[:])
            return out

        # ---------------- hops 1..R-1 (write bounce, allgather) ----------
        for r in range(r_hops - 1):
            for t in range(tiles):
                g = gather_tile(hfull[r], t)
                sf = tree_sum(g, int(Ks[t]))
                sT = transpose_sb(sf[:, :P], f"s{r}")
                ph = ptile("psB", P)
                nc.tensor.matmul(ph[:], lhsT=sT[:], rhs=gw[:, r * P:(r + 1) * P],
                                 start=True, stop=not nz["gcn_b"])
                if nz["gcn_b"]:
                    rank1(ph[:], ones[:], gb[:, r * P:(r + 1) * P])
                hn = sbh.tile([P, P], bf16, name="hn")
                nc.scalar.activation(hn[:], ph[:], Relu)
                store_h(bounce[r + 1], t, hn)
            allgather_split(r + 1)

        # ---------------- hop R + decoder + log_softmax ------------------
        # h3T = relu(gw2.T @ sT); zT = relu(dw1.T @ h3T); logits = zT.T @ dw2
        rl = r_hops - 1
        for t in range(tiles):
            g = gather_tile(hfull[rl], t)
            sf = tree_sum(g, int(Ks[t]))
            sT = transpose_sb(sf[:, :P], "s_last")
            ph3 = ptile("psB", P)
            nc.tensor.matmul(ph3[:], lhsT=gw[:, rl * P:(rl + 1) * P], rhs=sT[:],
                             start=True, stop=True)
            h3T = sbh.tile([P, P], bf16, name="h3T")
            if nz["gcn_b"]:
                nc.scalar.activation(h3T[:], ph3[:], Relu, bias=gb2T[:, 0:1])
            else:
                nc.scalar.activation(h3T[:], ph3[:], Relu)

            pz = ptile("psC", P)
            nc.tensor.matmul(pz[:], lhsT=dw1[:], rhs=h3T[:],
                             start=True, stop=True)
            zT = sbh.tile([P, P], bf16, name="zT")
            if nz["dec_b1"]:
                nc.scalar.activation(zT[:], pz[:], Relu, bias=db1T[:, 0:1])
            else:
                nc.scalar.activation(zT[:], pz[:], Relu)

            pl_t = ptile("psB", P)
            pl = pl_t[:, 0:ncls]
            nc.tensor.matmul(pl[:], lhsT=zT[:], rhs=dw2[:], start=True,
                             stop=not nz["dec_b2"])
            if nz["dec_b2"]:
                rank1(pl[:], ones[:], db2[:])

            # log_softmax pass 1: -max, exp-sum, stash logits
            nc.vector.tensor_reduce(nmx_all[:, t:t + 1], pl[:],
                                    axis=mybir.AxisListType.X,
                                    op=mybir.AluOpType.max, negate=True)
            nc.vector.tensor_copy(logits_all[:, t * ncls:(t + 1) * ncls], pl[:])
            nc.scalar.activation(ex_scratch[:], pl[:],
                                 mybir.ActivationFunctionType.Exp,
                                 bias=nmx_all[:, t:t + 1],
                                 accum_out=esum_all[:, t:t + 1])

        # log_softmax pass 2 in two halves: the first half's Ln/shift/output
        # runs while hop-R still gathers the later tiles.
        HT = tiles // 2
        for (a, b) in ((0, HT), (HT, tiles)):
            nc.scalar.activation(ln_all[:, a:b], esum_all[:, a:b],
                                 mybir.ActivationFunctionType.Ln)
            nc.vector.tensor_tensor(shift_all[:, a:b], nmx_all[:, a:b],
                                    ln_all[:, a:b],
                                    op=mybir.AluOpType.subtract)  # -max - ln
            for t in range(a, b):
                ot = sb.tile([P, ncls], f32, name="ot")
                nc.vector.tensor_tensor(
                    ot[:], logits_all[:, t * ncls:(t + 1) * ncls],
                    shift_all[:, t:t + 1].to_broadcast([P, ncls]),
                    op=mybir.AluOpType.add)
                nc.sync.dma_start(io["out"][t * P:(t + 1) * P, :], ot[:])


def _build_program(cfg):
    from concourse import bacc, mybir, tile

    f32 = mybir.dt.float32
    bf16 = mybir.dt.bfloat16
    i32 = mybir.dt.int32
    nc = bacc.Bacc("TRN2", target_bir_lowering=False, debug=False,
                   num_devices=cfg["W"])
    nl, sumk, h2, ncls, r_hops = cfg["NL"], cfg["SUMK"], cfg["H"], cfg["C"], cfg["R"]
    io = {
        "xT": nc.dram_tensor("xT", [P, nl], bf16, kind="ExternalInput").ap(),
        "idx": nc.dram_tensor("idx", [P, sumk], i32, kind="ExternalInput").ap(),
        "enc_w1": nc.dram_tensor("enc_w1", [P, h2], bf16, kind="ExternalInput").ap(),
        "enc_b1T": nc.dram_tensor("enc_b1T", [P, h2 // P], bf16, kind="ExternalInput").ap(),
        "enc_w2": nc.dram_tensor("enc_w2", [h2, P], bf16, kind="ExternalInput").ap(),
        "enc_b2": nc.dram_tensor("enc_b2", [1, P], bf16, kind="ExternalInput").ap(),
        "gcn_w": nc.dram_tensor("gcn_w", [r_hops * P, P], bf16, kind="ExternalInput").ap(),
        "gcn_b": nc.dram_tensor("gcn_b", [1, r_hops * P], bf16, kind="ExternalInput").ap(),
        "gcn_b2T": nc.dram_tensor("gcn_b2T", [P, 1], bf16, kind="ExternalInput").ap(),
        "dec_w1": nc.dram_tensor("dec_w1", [P, P], bf16, kind="ExternalInput").ap(),
        "dec_b1T": nc.dram_tensor("dec_b1T", [P, 1], bf16, kind="ExternalInput").ap(),
        "dec_w2": nc.dram_tensor("dec_w2", [P, ncls], bf16, kind="ExternalInput").ap(),
        "dec_b2": nc.dram_tensor("dec_b2", [1, ncls], bf16, kind="ExternalInput").ap(),
        "out": nc.dram_tensor("out", [nl, ncls], f32, kind="ExternalOutput").ap(),
    }
    with tile.TileContext(nc) as tc:
        _emit(tc, io, cfg)
    nc.compile()
    return nc


_CACHE = {}
LAST_RESULT = None


def _make_cfg(Ks, offs, sumk, nl, w, real_per_core, nonzero_bias):
    return dict(NL=nl, TILES=nl // P, SUMK=sumk, Ks=Ks, offs=offs,
                H=H, R=R, C=C, W=w, REAL_PER_CORE=real_per_core,
                NONZERO_BIAS=nonzero_bias)


def kernel(x, edge_index, enc_w1, enc_b1, enc_w2, enc_b2,
           gcn_w, gcn_b, dec_w1, dec_b1, dec_w2, dec_b2):
    global LAST_RESULT
    from concourse.bass_utils import run_bass_kernel_spmd

    nl = 12544
    x = np.asarray(x, dtype=np.float32)
    edge_index = np.asarray(edge_index)
    new_of_old, old_of_new, per_core_idx, Ks, offs, sumk = _build_plan(
        edge_index, N, W, nl)

    nonzero_bias = {
        "enc_b1": bool(np.any(np.asarray(enc_b1))),
        "enc_b2": bool(np.any(np.asarray(enc_b2))),
        "gcn_b": bool(np.any(np.asarray(gcn_b))),
        "dec_b1": bool(np.any(np.asarray(dec_b1))),
        "dec_b2": bool(np.any(np.asarray(dec_b2))),
    }
    key = ("prog", sumk, tuple(Ks.tolist()), tuple(sorted(nonzero_bias.items())))
    if key not in _CACHE:
        cfg = _make_cfg(Ks, offs, sumk, nl, W, N // W, nonzero_bias)
        _CACHE[key] = (_build_program(cfg), cfg)
    nc, cfg = _CACHE[key]

    # per-core inputs
    import ml_dtypes
    bf = ml_dtypes.bfloat16
    b1 = np.asarray(enc_b1, np.float32)
    weights = {
        "enc_w1": np.asarray(enc_w1, np.float32).astype(bf),
        "enc_b1T": np.ascontiguousarray(
            b1.reshape(H // P, P).T).astype(bf),          # [128, 2]
        "enc_w2": np.asarray(enc_w2, np.float32).astype(bf),
        "enc_b2": np.asarray(enc_b2, np.float32).reshape(1, D).astype(bf),
        "gcn_w": np.asarray(gcn_w, np.float32).reshape(R * D, D).astype(bf),
        "gcn_b": np.asarray(gcn_b, np.float32).reshape(1, R * D).astype(bf),
        "gcn_b2T": np.asarray(gcn_b, np.float32).reshape(R, D)[R - 1]
            .reshape(D, 1).astype(bf),
        "dec_w1": np.asarray(dec_w1, np.float32).astype(bf),
        "dec_b1T": np.asarray(dec_b1, np.float32).reshape(D, 1).astype(bf),
        "dec_w2": np.asarray(dec_w2, np.float32).astype(bf),
        "dec_b2": np.asarray(dec_b2, np.float32).reshape(1, C).astype(bf),
    }
    in_maps = []
    for c in range(W):
        ids = old_of_new[c * nl:(c + 1) * nl]
        xs = np.zeros((nl, F), np.float32)
        real = ids >= 0
        xs[real] = x[ids[real]]
        im = dict(weights)
        im["xT"] = np.ascontiguousarray(xs.T).astype(bf)
        im["idx"] = per_core_idx[c]
        in_maps.append(im)

    res = run_bass_kernel_spmd(
        nc, in_maps, core_ids=list(range(W)),
        trace=bool(int(os.environ.get("KERNEL_TRACE", "0"))))
    LAST_RESULT = res

    out = np.empty((N, C), np.float32)
    for c in range(W):
        ids = old_of_new[c * nl:(c + 1) * nl]
        real = ids >= 0
        out[ids[real]] = np.asarray(res.results[c]["out"])[real]
    return out


# revision 13
# speedup vs baseline: 1.1685x; 1.1685x over previous
"""Trainium2 Bass kernel for 3-hop GCN (nn_GNN_70978629534135).

Strategy (8 NeuronCores, SPMD):
  - Nodes are permuted by in-degree (descending) and snake-dealt across the 8
    cores so every core owns NL=12544 slots (12500 real + 44 zero "fake" pads)
    with a near-identical degree profile.
  - All edge indices are remapped into the permuted id space on the host, and
    partitioned by destination core.  For each 128-node tile the in-edges are
    padded to the tile-max degree K_t and stored as a [128, K_t] int32 gather
    table (pad slots point at a row that is always zero).
  - Since matmul distributes over the neighbor sum, each hop gathers raw h
    (not h@W): per-k indirect DMAs (128 rows each, the HW limit for dynamic
    offsets), one wide strided DVE op per tree level for the neighbor sum,
    PE transpose, matmul, ReLU.
  - Encoder/decoder matmul chains are arranged transposed (lhsT=weight) so no
    PE transposes are needed there; biases that are all-zero on the host are
    elided entirely (this model's biases are structurally zero).
  - log_softmax: per-tile max/exp (Exp table stays loaded), one batched Ln at
    the end, then a broadcast-add pass writes the output.
  - AllGathers are split in half so the first half overlaps the tail of each
    hop's compute.

kernel(**inputs) takes the FULL unsharded inputs and returns the FULL output.
"""

import os
import numpy as np

N, E, F, H, D, R, C, W = 100_000, 600_000, 128, 256, 128, 3, 40, 8
P = 128


# ---------------------------------------------------------------- host prep
def _build_plan(edge_index, n, w, nl):
    tiles = nl // P
    half = (tiles // 2) * P  # AG split point (slots per core)
    src = edge_index[0].astype(np.int64)
    dst = edge_index[1].astype(np.int64)
    deg = np.bincount(dst, minlength=n)
    order = np.argsort(-deg, kind="stable")
    new_of_old = np.empty(n, dtype=np.int64)
    pos = np.arange(n)
    new_of_old[order] = (pos % w) * nl + (pos // w)
    src_n = new_of_old[src]
    dst_n = new_of_old[dst]

    # hfull row layout is split for the two-phase AllGather:
    #   rows [0, w*half)        = slot s < half of core c at row c*half + s
    #   rows [w*half, w*nl)     = slot s >= half at w*half + c*(nl-half) + (s-half)
    def hrow(gid):
        c, s = gid // nl, gid % nl
        return np.where(s < half, c * half + s,
                        w * half + c * (nl - half) + (s - half))
    src_h = hrow(src_n)
    zrow = int(hrow(np.array([nl - 1]))[0])  # core 0's last slot: fake (zero)

    per_core = []
    Ks = np.zeros(tiles, dtype=np.int64)
    csr = []
    for c in range(w):
        m = (dst_n >= c * nl) & (dst_n < (c + 1) * nl)
        d_loc = dst_n[m] - c * nl
        s_glob = src_h[m]
        o = np.argsort(d_loc, kind="stable")
        d_loc, s_glob = d_loc[o], s_glob[o]
        counts = np.bincount(d_loc, minlength=nl)
        rowptr = np.concatenate([[0], np.cumsum(counts)])
        csr.append((d_loc, s_glob, rowptr))
        np.maximum(Ks, counts.reshape(tiles, P).max(axis=1), out=Ks)

    offs = np.concatenate([[0], np.cumsum(Ks)]).astype(np.int64)
    sumk = int(Ks.sum())
    for c in range(w):
        d_loc, s_glob, rowptr = csr[c]
        rank = np.arange(len(d_loc)) - rowptr[d_loc]
        col = offs[d_loc // P] + rank
        idx = np.full((P, sumk), zrow, dtype=np.int32)
        idx[d_loc % P, col] = s_glob
        per_core.append(idx)

    old_of_new = np.full(w * nl, -1, dtype=np.int64)
    old_of_new[new_of_old] = np.arange(n)
    return new_of_old, old_of_new, per_core, Ks.astype(int), offs, sumk


# ------------------------------------------------------------- device program
def _emit(tc, io, cfg):
    import concourse.bass as bass
    from concourse import mybir
    from concourse.masks import make_identity

    nc = tc.nc
    f32 = mybir.dt.float32
    bf16 = mybir.dt.bfloat16
    nl, tiles, sumk = cfg["NL"], cfg["TILES"], cfg["SUMK"]
    Ks, offs = cfg["Ks"], cfg["offs"]
    h2, r_hops = cfg["H"], cfg["R"]
    ncls = cfg["C"]
    nz = cfg["NONZERO_BIAS"]  # dict name -> bool
    AG_GROUPS = [list(range(cfg["W"]))]
    ADD = mybir.AluOpType.add
    Relu = mybir.ActivationFunctionType.Relu
    HALF = (tiles // 2) * P  # AG split point (rows)

    def rank1(psum_ap, ones, bias_ap, stop=True):
        nc.tensor.matmul(psum_ap, lhsT=ones, rhs=bias_ap, start=False, stop=stop)

    with tc.tile_pool(name="const", bufs=1) as cp, \
         tc.tile_pool(name="sb", bufs=3) as sb, \
         tc.tile_pool(name="sb_g", bufs=6) as sbg, \
         tc.tile_pool(name="sb_h", bufs=3) as sbh, \
         tc.tile_pool(name="ps", bufs=2, space="PSUM") as ps:

        def ptile(tag, width):
            return ps.tile([P, width], f32, space="PSUM", tag=tag, name=tag)

        ident = cp.tile([P, P], f32)
        make_identity(nc, ident[:])
        zeros128 = cp.tile([P, P], bf16)
        nc.gpsimd.memset(zeros128[:], 0.0)
        real_end = cfg["REAL_PER_CORE"]

        def store_h(dst_dram, t, h_tile):
            lo, hi = t * P, (t + 1) * P
            real_hi = min(hi, real_end)
            if real_hi > lo:
                nc.sync.dma_start(dst_dram[lo:real_hi, :], h_tile[0:real_hi - lo, :])
            if hi > real_hi:
                nc.sync.dma_start(dst_dram[real_hi:hi, :],
                                  zeros128[0:hi - real_hi, :])
        ones = cp.tile([1, P], bf16)
        nc.gpsimd.memset(ones[:], 1.0)

        # resident weights
        w1 = cp.tile([P, h2], bf16)          # enc_w1 [F, H]
        nc.sync.dma_start(w1[:], io["enc_w1"][:])
        w2 = cp.tile([P, h2], bf16)          # enc_w2 [H, D] -> [:, j*128:] = rows j
        for j in range(h2 // P):
            nc.sync.dma_start(w2[:, j * P:(j + 1) * P], io["enc_w2"][j * P:(j + 1) * P, :])
        gw = cp.tile([P, r_hops * P], bf16)  # gcn_w stacked [R*D, D]
        for r in range(r_hops):
            nc.sync.dma_start(gw[:, r * P:(r + 1) * P], io["gcn_w"][r * P:(r + 1) * P, :])
        dw1 = cp.tile([P, P], bf16)
        nc.sync.dma_start(dw1[:], io["dec_w1"][:])
        dw2 = cp.tile([P, ncls], bf16)
        nc.sync.dma_start(dw2[:], io["dec_w2"][:])
        if nz["enc_b1"]:
            b1T = cp.tile([P, h2 // P], bf16)
            nc.sync.dma_start(b1T[:], io["enc_b1T"][:])
        if nz["enc_b2"]:
            b2 = cp.tile([1, P], bf16)
            nc.sync.dma_start(b2[:], io["enc_b2"][:])
        if nz["gcn_b"]:
            gb = cp.tile([1, r_hops * P], bf16)
            nc.sync.dma_start(gb[:], io["gcn_b"][:])
            gb2T = cp.tile([P, 1], bf16)
            nc.sync.dma_start(gb2T[:], io["gcn_b2T"][:])
        if nz["dec_b1"]:
            db1T = cp.tile([P, 1], bf16)
            nc.sync.dma_start(db1T[:], io["dec_b1T"][:])
        if nz["dec_b2"]:
            db2 = cp.tile([1, ncls], bf16)
            nc.sync.dma_start(db2[:], io["dec_b2"][:])

        idx_sb = cp.tile([P, sumk], mybir.dt.int32)
        nc.sync.dma_start(idx_sb[:], io["idx"][:])

        # decoder softmax state (persists across tiles)
        logits_all = cp.tile([P, tiles * ncls], f32)
        nmx_all = cp.tile([P, tiles], f32)
        esum_all = cp.tile([P, tiles], f32)
        ln_all = cp.tile([P, tiles], f32)
        shift_all = cp.tile([P, tiles], f32)
        ex_scratch = cp.tile([P, ncls], f32)

        # internal DRAM: allgather bounce + full tables
        bounce = [nc.dram_tensor(f"bounce{r}", [nl, P], bf16, kind="Internal")
                  for r in range(r_hops)]
        hfull = [nc.dram_tensor(f"hfull{r}", [cfg["W"] * nl, P], bf16,
                                kind="Internal", addr_space="Shared")
                 for r in range(r_hops)]

        def allgather_split(r):
            # hfull row layout: [w*HALF first-half rows | w*(nl-HALF) rest]
            # so each half-AllGather writes one contiguous block; the first
            # half is issued as soon as rows [0, HALF) of bounce are stored.
            nc.gpsimd.collective_compute(
                "AllGather", mybir.AluOpType.bypass,
                ins=[bounce[r][0:HALF, :]],
                outs=[hfull[r][0:cfg["W"] * HALF, :]],
                replica_groups=AG_GROUPS)
            nc.gpsimd.collective_compute(
                "AllGather", mybir.AluOpType.bypass,
                ins=[bounce[r][HALF:nl, :]],
                outs=[hfull[r][cfg["W"] * HALF:cfg["W"] * nl, :]],
                replica_groups=AG_GROUPS)

        # ---------------- encoder ----------------
        # z1T = relu(w1.T @ xT [+ b1T]) computed directly in transposed layout;
        # h0 = z1 @ w2 [+ b2] via lhsT=z1T halves.  Four dst tiles are batched
        # per pass (512-wide rhs) and the four h0 tiles ship in ONE store DMA
        # so the Sync queue doesn't pace this phase.
        TB = 4
        full = (min(tiles, real_end // P) // TB) * TB  # quad-batched region
        for t0 in range(0, full, TB):
            wb = TB * P
            xT_t = sb.tile([P, TB * P], bf16, name="xT_t")
            nc.sync.dma_start(xT_t[:], io["xT"][:, t0 * P:t0 * P + wb])
            p1a = ptile("psA", TB * P)
            p1b = ptile("psB", TB * P)
            nc.tensor.matmul(p1a[:], lhsT=w1[:, 0:P], rhs=xT_t[:],
                             start=True, stop=True)
            nc.tensor.matmul(p1b[:], lhsT=w1[:, P:h2], rhs=xT_t[:],
                             start=True, stop=True)
            z1a = sbh.tile([P, TB * P], bf16, name="z1a")
            z1b = sbh.tile([P, TB * P], bf16, name="z1b")
            if nz["enc_b1"]:
                nc.scalar.activation(z1a[:], p1a[:], Relu, bias=b1T[:, 0:1])
                nc.scalar.activation(z1b[:], p1b[:], Relu, bias=b1T[:, 1:2])
            else:
                nc.scalar.activation(z1a[:], p1a[:], Relu)
                nc.scalar.activation(z1b[:], p1b[:], Relu)
            h0q = sbh.tile([P, TB * P], bf16, name="h0q")
            for i in range(TB):
                p2 = ptile("psC", P)
                nc.tensor.matmul(p2[:], lhsT=z1a[:, i * P:(i + 1) * P],
                                 rhs=w2[:, 0:P], start=True, stop=False)
                nc.tensor.matmul(p2[:], lhsT=z1b[:, i * P:(i + 1) * P],
                                 rhs=w2[:, P:h2],
                                 start=False, stop=not nz["enc_b2"])
                if nz["enc_b2"]:
                    rank1(p2[:], ones[:], b2[:])
                nc.vector.tensor_copy(h0q[:, i * P:(i + 1) * P], p2[:])
            # one DMA for all TB tiles: DRAM rows (j*128+i) <- h0q[i, j*128:]
            dst = bounce[0][t0 * P:(t0 + TB) * P, :].rearrange(
                "(j i) d -> i j d", j=TB)
            nc.sync.dma_start(dst, h0q[:].rearrange("p (j d) -> p j d", j=TB))
        for t in range(full, tiles):
            xT_t = sb.tile([P, P], bf16, name="xT_t1")
            nc.sync.dma_start(xT_t[:], io["xT"][:, t * P:(t + 1) * P])
            p1a = ptile("psA", P)
            p1b = ptile("psB", P)
            nc.tensor.matmul(p1a[:, 0:P], lhsT=w1[:, 0:P], rhs=xT_t[:],
                             start=True, stop=True)
            nc.tensor.matmul(p1b[:, 0:P], lhsT=w1[:, P:h2], rhs=xT_t[:],
                             start=True, stop=True)
            z1a = sbh.tile([P, P], bf16, name="z1a1")
            z1b = sbh.tile([P, P], bf16, name="z1b1")
            if nz["enc_b1"]:
                nc.scalar.activation(z1a[:], p1a[:, 0:P], Relu, bias=b1T[:, 0:1])
                nc.scalar.activation(z1b[:], p1b[:, 0:P], Relu, bias=b1T[:, 1:2])
            else:
                nc.scalar.activation(z1a[:], p1a[:, 0:P], Relu)
                nc.scalar.activation(z1b[:], p1b[:, 0:P], Relu)
            p2 = ptile("psC", P)
            nc.tensor.matmul(p2[:], lhsT=z1a[:], rhs=w2[:, 0:P],
                             start=True, stop=False)
            nc.tensor.matmul(p2[:], lhsT=z1b[:], rhs=w2[:, P:h2],
                             start=False, stop=not nz["enc_b2"])
            if nz["enc_b2"]:
                rank1(p2[:], ones[:], b2[:])
            h0 = sbh.tile([P, P], bf16, name="h0")
            nc.vector.tensor_copy(h0[:], p2[:])
            store_h(bounce[0], t, h0)
        allgather_split(0)

        # ------------- gather + wide tree-sum ---------------------
        def gather_tile(src_full, t):
            K = int(Ks[t])
            off = int(offs[t])
            g = sbg.tile([P, max(K, 1) * P], bf16, name="g")
            for k in range(K):
                nc.gpsimd.indirect_dma_start(
                    out=g[:, k * P:(k + 1) * P], out_offset=None, in_=src_full[:],
                    in_offset=bass.IndirectOffsetOnAxis(
                        ap=idx_sb[:, off + k:off + k + 1], axis=0))
            return g

        def tree_sum(g, K):
            """Sum K gathered [128,128] blocks of g -> f32 tile [128, 128]."""
            sf = sbh.tile([P, max((K + 1) // 2, 1) * P], f32, name="sf")
            if K == 0:
                nc.vector.memset(sf[:, :P], 0.0)
                return sf
            if K == 1:
                nc.vector.tensor_copy(sf[:, :P], g[:, 0:P])
                return sf
            half = K // 2
            # level 1: one strided op: pairs (2j, 2j+1) -> sf (bf16+bf16->f32)
            gpair = g[:, 0:2 * half * P].rearrange(
                "p (k two j) -> p two k j", two=2, j=P)
            sf3 = sf[:, 0:half * P].rearrange("p (k j) -> p k j", j=P)
            nc.vector.tensor_tensor(sf3, gpair[:, 0], gpair[:, 1], op=ADD)
            if K % 2:
                nc.vector.tensor_copy(sf[:, half * P:(half + 1) * P],
                                      g[:, (K - 1) * P:K * P])
            kk = (K + 1) // 2
            while kk > 1:
                if kk % 2:
                    nc.vector.tensor_tensor(
                        sf[:, :P], sf[:, :P], sf[:, (kk - 1) * P:kk * P], op=ADD)
                    kk -= 1
                half2 = kk // 2
                nc.vector.tensor_tensor(
                    sf[:, :half2 * P], sf[:, :half2 * P],
                    sf[:, half2 * P:kk * P], op=ADD)
                kk = half2
            return sf

        def transpose_sb(src_ap, name):
            pt = ptile("psA", P)
            nc.tensor.transpose(pt[:], src_ap, ident[:])
            out = sbh.tile([P, P], bf16, name=f"sb_{name}")
            nc.vector.tensor_copy(out[:], pt[:])
            return out

        # ---------------- hops 1..R-1 (write bounce, allgather) ----------
        for r in range(r_hops - 1):
            for t in range(tiles):
                g = gather_tile(hfull[r], t)
                sf = tree_sum(g, int(Ks[t]))
                sT = transpose_sb(sf[:, :P], f"s{r}")
                ph = ptile("psB", P)
                nc.tensor.matmul(ph[:], lhsT=sT[:], rhs=gw[:, r * P:(r + 1) * P],
                                 start=True, stop=not nz["gcn_b"])
                if nz["gcn_b"]:
                    rank1(ph[:], ones[:], gb[:, r * P:(r + 1) * P])
                hn = sbh.tile([P, P], bf16, name="hn")
                nc.scalar.activation(hn[:], ph[:], Relu)
                store_h(bounce[r + 1], t, hn)
            allgather_split(r + 1)

        # ---------------- hop R + decoder + log_softmax ------------------
        # h3T = relu(gw2.T @ sT); zT = relu(dw1.T @ h3T); logits = zT.T @ dw2
        rl = r_hops - 1
        for t in range(tiles):
            g = gather_tile(hfull[rl], t)
            sf = tree_sum(g, int(Ks[t]))
            sT = transpose_sb(sf[:, :P], "s_last")
            ph3 = ptile("psB", P)
            nc.tensor.matmul(ph3[:], lhsT=gw[:, rl * P:(rl + 1) * P], rhs=sT[:],
                             start=True, stop=True)
            h3T = sbh.tile([P, P], bf16, name="h3T")
            if nz["gcn_b"]:
                nc.scalar.activation(h3T[:], ph3[:], Relu, bias=gb2T[:, 0:1])
            else:
                nc.scalar.activation(h3T[:], ph3[:], Relu)

            pz = ptile("psC", P)
            nc.tensor.matmul(pz[:], lhsT=dw1[:], rhs=h3T[:],
                             start=True, stop=True)
            zT = sbh.tile([P, P], bf16, name="zT")
            if nz["dec_b1"]:
                nc.scalar.activation(zT[:], pz[:], Relu, bias=db1T[:, 0:1])
            else:
                nc.scalar.activation(zT[:], pz[:], Relu)

            pl_t = ptile("psB", P)
            pl = pl_t[:, 0:ncls]
            nc.tensor.matmul(pl[:], lhsT=zT[:], rhs=dw2[:], start=True,
                             stop=not nz["dec_b2"])
            if nz["dec_b2"]:
                rank1(pl[:], ones[:], db2[:])

            # log_softmax pass 1: -max, exp-sum, stash logits
            nc.vector.tensor_reduce(nmx_all[:, t:t + 1], pl[:],
                                    axis=mybir.AxisListType.X,
                                    op=mybir.AluOpType.max, negate=True)
            nc.vector.tensor_copy(logits_all[:, t * ncls:(t + 1) * ncls], pl[:])
            nc.scalar.activation(ex_scratch[:], pl[:],
                                 mybir.ActivationFunctionType.Exp,
                                 bias=nmx_all[:, t:t + 1],
                                 accum_out=esum_all[:, t:t + 1])

        # log_softmax pass 2 in two halves: the first half's Ln/shift/output
        # runs while hop-R still gathers the later tiles.  Output ships in
        # 4-tile batched DMAs to keep the tail short.
        HT = tiles // 2
        OB = 4
        for (a, b) in ((0, HT), (HT, tiles)):
            nc.scalar.activation(ln_all[:, a:b], esum_all[:, a:b],
                                 mybir.ActivationFunctionType.Ln)
            nc.vector.tensor_tensor(shift_all[:, a:b], nmx_all[:, a:b],
                                    ln_all[:, a:b],
                                    op=mybir.AluOpType.subtract)  # -max - ln
            t = a
            while t < b:
                ob = min(OB, b - t)
                ot = sb.tile([P, OB * ncls], f32, name="ot")
                for i in range(ob):
                    nc.vector.tensor_tensor(
                        ot[:, i * ncls:(i + 1) * ncls],
                        logits_all[:, (t + i) * ncls:(t + i + 1) * ncls],
                        shift_all[:, t + i:t + i + 1].to_broadcast([P, ncls]),
                        op=mybir.AluOpType.add)
                dst = io["out"][t * P:(t + ob) * P, :].rearrange(
                    "(j i) d -> i j d", j=ob)
                nc.sync.dma_start(
                    dst, ot[:, 0:ob * ncls].rearrange("p (j d) -> p j d", j=ob))
                t += ob


def _build_program(cfg):
    from concourse import bacc, mybir, tile

    f32 = mybir.dt.float32
    bf16 = mybir.dt.bfloat16
    i32 = mybir.dt.int32
    nc = bacc.Bacc("TRN2", target_bir_lowering=False, debug=False,
                   num_devices=cfg["W"])
    nl, sumk, h2, ncls, r_hops = cfg["NL"], cfg["SUMK"], cfg["H"], cfg["C"], cfg["R"]
    io = {
        "xT": nc.dram_tensor("xT", [P, nl], bf16, kind="ExternalInput").ap(),
        "idx": nc.dram_tensor("idx", [P, sumk], i32, kind="ExternalInput").ap(),
        "enc_w1": nc.dram_tensor("enc_w1", [P, h2], bf16, kind="ExternalInput").ap(),
        "enc_b1T": nc.dram_tensor("enc_b1T", [P, h2 // P], bf16, kind="ExternalInput").ap(),
        "enc_w2": nc.dram_tensor("enc_w2", [h2, P], bf16, kind="ExternalInput").ap(),
        "enc_b2": nc.dram_tensor("enc_b2", [1, P], bf16, kind="ExternalInput").ap(),
        "gcn_w": nc.dram_tensor("gcn_w", [r_hops * P, P], bf16, kind="ExternalInput").ap(),
        "gcn_b": nc.dram_tensor("gcn_b", [1, r_hops * P], bf16, kind="ExternalInput").ap(),
        "gcn_b2T": nc.dram_tensor("gcn_b2T", [P, 1], bf16, kind="ExternalInput").ap(),
        "dec_w1": nc.dram_tensor("dec_w1", [P, P], bf16, kind="ExternalInput").ap(),
        "dec_b1T": nc.dram_tensor("dec_b1T", [P, 1], bf16, kind="ExternalInput").ap(),
        "dec_w2": nc.dram_tensor("dec_w2", [P, ncls], bf16, kind="ExternalInput").ap(),
        "dec_b2": nc.dram_tensor("dec_b2", [1, ncls], bf16, kind="ExternalInput").ap(),
        "out": nc.dram_tensor("out", [nl, ncls], f32, kind="ExternalOutput").ap(),
    }
    with tile.TileContext(nc) as tc:
        _emit(tc, io, cfg)
    nc.compile()
    return nc


_CACHE = {}
LAST_RESULT = None


def _make_cfg(Ks, offs, sumk, nl, w, real_per_core, nonzero_bias):
    return dict(NL=nl, TILES=nl // P, SUMK=sumk, Ks=Ks, offs=offs,
                H=H, R=R, C=C, W=w, REAL_PER_CORE=real_per_core,
                NONZERO_BIAS=nonzero_bias)


def kernel(x, edge_index, enc_w1, enc_b1, enc_w2, enc_b2,
           gcn_w, gcn_b, dec_w1, dec_b1, dec_w2, dec_b2):
    global LAST_RESULT
    from concourse.bass_utils import run_bass_kernel_spmd

    nl = 12544
    x = np.asarray(x, dtype=np.float32)
    edge_index = np.asarray(edge_index)
    new_of_old, old_of_new, per_core_idx, Ks, offs, sumk = _build_plan(
        edge_index, N, W, nl)

    nonzero_bias = {
        "enc_b1": bool(np.any(np.asarray(enc_b1))),
        "enc_b2": bool(np.any(np.asarray(enc_b2))),
        "gcn_b": bool(np.any(np.asarray(gcn_b))),
        "dec_b1": bool(np.any(np.asarray(dec_b1))),
        "dec_b2": bool(np.any(np.asarray(dec_b2))),
    }
    key = ("prog", sumk, tuple(Ks.tolist()), tuple(sorted(nonzero_bias.items())))
    if key not in _CACHE:
        cfg = _make_cfg(Ks, offs, sumk, nl, W, N // W, nonzero_bias)
        _CACHE[key] = (_build_program(cfg), cfg)
    nc, cfg = _CACHE[key]

    # per-core inputs
    import ml_dtypes
    bf = ml_dtypes.bfloat16
    b1 = np.asarray(enc_b1, np.float32)
    weights = {
        "enc_w1": np.asarray(enc_w1, np.float32).astype(bf),
        "enc_b1T": np.ascontiguousarray(
            b1.reshape(H // P, P).T).astype(bf),          # [128, 2]
        "enc_w2": np.asarray(enc_w2, np.float32).astype(bf),
        "enc_b2": np.asarray(enc_b2, np.float32).reshape(1, D).astype(bf),
        "gcn_w": np.asarray(gcn_w, np.float32).reshape(R * D, D).astype(bf),
        "gcn_b": np.asarray(gcn_b, np.float32).reshape(1, R * D).astype(bf),
        "gcn_b2T": np.asarray(gcn_b, np.float32).reshape(R, D)[R - 1]
            .reshape(D, 1).astype(bf),
        "dec_w1": np.asarray(dec_w1, np.float32).astype(bf),
        "dec_b1T": np.asarray(dec_b1, np.float32).reshape(D, 1).astype(bf),
        "dec_w2": np.asarray(dec_w2, np.float32).astype(bf),
        "dec_b2": np.asarray(dec_b2, np.float32).reshape(1, C).astype(bf),
    }
    in_maps = []
    for c in range(W):
        ids = old_of_new[c * nl:(c + 1) * nl]
        xs = np.zeros((nl, F), np.float32)
        real = ids >= 0
        xs[real] = x[ids[real]]
        im = dict(weights)
        im["xT"] = np.ascontiguousarray(xs.T).astype(bf)
        im["idx"] = per_core_idx[c]
        in_maps.append(im)

    res = run_bass_kernel_spmd(
        nc, in_maps, core_ids=list(range(W)),
        trace=bool(int(os.environ.get("KERNEL_TRACE", "0"))))
    LAST_RESULT = res

    out = np.empty((N, C), np.float32)
    for c in range(W):
        ids = old_of_new[c * nl:(c + 1) * nl]
        real = ids >= 0
        out[ids[real]] = np.asarray(res.results[c]["out"])[real]
    return out
